# revision 1
# baseline (speedup 1.0000x reference)
"""Causal single-head attention (B=4, T=4096, C=1024, D=64) on 8 NeuronCores.

Sharding: core c = (batch b = c % 4, half h = c // 4).
Each core handles ALL queries of its batch, but only its half of the key
blocks (256-token key blocks with block index ≡ h mod 2).  This makes the
program identical on every core (pure SPMD, no control flow); cores differ
only in input data.  Each core emits unnormalized partial results
U^T = [V|1]^T @ exp(S^T) per query supertile; the host combines the two
halves per batch: O = (U0 + U1)[:64] / (U0 + U1)[64].

On-chip dataflow (all bf16 except PSUM/f32 accumulators):
  xq^T [C, T]   -> Q^T [64, T]          (matmul, C-tiled accumulation)
  xk^T [C, T/2] -> K^T, V^T [65, T/2]   (V^T row 64 = ones, for row-sums)
  V' [128, 65] per key tile              (PE transpose of V^T)
  S^T [128k, 512q] = K_tile @ Q^T        (matmul, contraction over D=64)
  P = exp(S^T/8) * causal_mask           (ACT exp from PSUM, DVE mask mul)
  U^T [65, 512] += V'_j^T @ P_j          (matmul, contraction over 128 keys)
"""
import sys
import numpy as np
import ml_dtypes

if "/opt/trn_rl_repo" not in sys.path:
    sys.path.insert(0, "/opt/trn_rl_repo")

import concourse.bacc as bacc
import concourse.mybir as mybir
from concourse import tile
from concourse import bass_utils

bf16 = mybir.dt.bfloat16
f32 = mybir.dt.float32
BF = ml_dtypes.bfloat16

B, T, C, D = 4, 4096, 1024, 64
NST = 8          # query supertiles per batch (512 queries each)
STQ = 512
TK = T // 2      # key tokens per core
NKT = TK // 128  # local 128-key tiles per core (16)
NC_ = C // 128   # 8 c-tiles

_CACHE = {}


def _build():
    nc = bacc.Bacc(None, target_bir_lowering=False, debug=False, num_devices=8)

    xq = nc.dram_tensor("xq", [C, T], bf16, kind="ExternalInput")
    xk = nc.dram_tensor("xk", [C, TK], bf16, kind="ExternalInput")
    w = nc.dram_tensor("w", [C, 192], bf16, kind="ExternalInput")   # Wq|Wk|Wv
    msk = nc.dram_tensor("msk", [256, STQ], bf16, kind="ExternalInput")
    idn = nc.dram_tensor("idn", [65, 65], bf16, kind="ExternalInput")
    out = nc.dram_tensor("out", [65, T], f32, kind="ExternalOutput")

    with tile.TileContext(nc) as tc:
        with tc.tile_pool(name="sb", bufs=1) as sb, \
             tc.tile_pool(name="pp", bufs=3) as pp, \
             tc.tile_pool(name="ps", bufs=2, space="PSUM") as ps:

            # ---- resident inputs ----
            xq_t = [sb.tile([128, T], bf16, tag=f"xq{c}", name=f"xq{c}")
                    for c in range(NC_)]
            xk_t = [sb.tile([128, TK], bf16, tag=f"xk{c}", name=f"xk{c}")
                    for c in range(NC_)]
            w_t = [sb.tile([128, 192], bf16, tag=f"w{c}", name=f"w{c}")
                   for c in range(NC_)]
            msk_t = sb.tile([128, 2 * STQ], bf16, tag="msk")
            idn_t = sb.tile([65, 65], bf16, tag="idn")
            for c in range(NC_):
                nc.sync.dma_start(xq_t[c][:], xq[128 * c:128 * (c + 1), :])
                nc.sync.dma_start(xk_t[c][:], xk[128 * c:128 * (c + 1), :])
                nc.sync.dma_start(w_t[c][:], w[128 * c:128 * (c + 1), :])
            nc.sync.dma_start(msk_t[:, 0:STQ], msk[0:128, :])
            nc.sync.dma_start(msk_t[:, STQ:2 * STQ], msk[128:256, :])
            nc.sync.dma_start(idn_t[:], idn[:])

            # ---- persistent intermediates ----
            qT = sb.tile([64, T], bf16, tag="qT")
            kT = sb.tile([64, TK], bf16, tag="kT")
            vT = sb.tile([65, TK], bf16, tag="vT")   # row 64 = ones
            vP = sb.tile([128, NKT * 65], bf16, tag="vP")  # V' tiles

            nc.vector.memset(vT[64:65, :], 1.0)

            # ---- projections ----
            # Q^T over all tokens, 512-wide chunks
            for st in range(NST):
                acc = ps.tile([64, STQ], f32, tag="work")
                for c in range(NC_):
                    nc.tensor.matmul(acc[:], w_t[c][:, 0:64],
                                     xq_t[c][:, STQ * st:STQ * (st + 1)],
                                     start=(c == 0), stop=(c == NC_ - 1))
                nc.vector.tensor_copy(qT[:, STQ * st:STQ * (st + 1)], acc[:])

            # K^T / V^T over local key tokens, 256-wide chunks
            for blk in range(TK // 256):
                sl = slice(256 * blk, 256 * (blk + 1))
                acck = ps.tile([64, 256], f32, tag="work")
                for c in range(NC_):
                    nc.tensor.matmul(acck[:], w_t[c][:, 64:128], xk_t[c][:, sl],
                                     start=(c == 0), stop=(c == NC_ - 1))
                nc.vector.tensor_copy(kT[:, sl], acck[:])
                accv = ps.tile([64, 256], f32, tag="work")
                for c in range(NC_):
                    nc.tensor.matmul(accv[:], w_t[c][:, 128:192], xk_t[c][:, sl],
                                     start=(c == 0), stop=(c == NC_ - 1))
                nc.vector.tensor_copy(vT[0:64, sl], accv[:])

            # V' tiles: transpose V^T (incl. ones row) per 128-key tile
            for j in range(NKT):
                tp = ps.tile([128, 65], bf16, tag="work")
                nc.tensor.transpose(tp[:], vT[:, 128 * j:128 * (j + 1)], idn_t[:])
                nc.vector.tensor_copy(vP[:, 65 * j:65 * (j + 1)], tp[:])

            # ---- attention ----
            for st in range(NST):
                qsl = slice(STQ * st, STQ * (st + 1))
                n = 2 * (st + 1)          # local key tiles for this supertile
                u = ps.tile([65, STQ], f32, tag="u")
                for j0 in range(0, n, 2):
                    s2 = ps.tile([128, 2 * STQ], f32, tag="s")
                    p2 = pp.tile([128, 2 * STQ], bf16, tag="p")
                    for d in range(2):
                        j = j0 + d
                        nc.tensor.matmul(s2[:, STQ * d:STQ * (d + 1)],
                                         kT[:, 128 * j:128 * (j + 1)],
                                         qT[:, qsl], start=True, stop=True)
                    nc.scalar.activation(p2[:], s2[:],
                                         mybir.ActivationFunctionType.Exp,
                                         scale=0.125)
                    if j0 == n - 2:  # diagonal pair -> causal masks
                        nc.vector.tensor_mul(p2[:], p2[:], msk_t[:])
                    for d in range(2):
                        j = j0 + d
                        nc.tensor.matmul(u[:], vP[:, 65 * j:65 * (j + 1)],
                                         p2[:, STQ * d:STQ * (d + 1)],
                                         start=(j == 0), stop=(j == n - 1))
                u_sb = pp.tile([65, STQ], f32, tag="u_sb")
                nc.vector.tensor_copy(u_sb[:], u[:])
                nc.sync.dma_start(out[:, qsl], u_sb[:])

    nc.compile()
    return nc


def _get_nc():
    if "nc" not in _CACHE:
        _CACHE["nc"] = _build()
    return _CACHE["nc"]


def kernel(x, Wq, Wk, Wv, _trace=False):
    x = np.asarray(x)
    nc = _get_nc()

    xT = np.ascontiguousarray(x.transpose(0, 2, 1)).astype(BF)   # [B, C, T]
    w = np.concatenate([Wq, Wk, Wv], axis=1).astype(BF)          # [C, 192]
    idn = np.eye(65, dtype=BF)

    j = np.arange(128)[:, None]
    i = np.arange(STQ)[None, :]
    masks = {}
    for h in range(2):
        m0 = (j <= i - 256 * h).astype(BF)
        m1 = (j <= i - 256 * h - 128).astype(BF)
        masks[h] = np.concatenate([m0, m1], axis=0)

    # key-token selector: 256-blocks with block index ≡ h (mod 2)
    tok = np.arange(T)
    keysel = {h: ((tok // 256) % 2 == h) for h in range(2)}

    in_maps = []
    for c in range(8):
        b, h = c % 4, c // 4
        in_maps.append({
            "xq": xT[b],
            "xk": np.ascontiguousarray(xT[b][:, keysel[h]]),
            "w": w,
            "msk": masks[h],
            "idn": idn,
        })

    res = bass_utils.run_bass_kernel_spmd(nc, in_maps, core_ids=list(range(8)),
                                          trace=_trace)
    _CACHE["last_results"] = res

    O = np.empty((B, T, D), dtype=np.float32)
    for b in range(B):
        U = res.results[b]["out"] + res.results[b + 4]["out"]    # [65, T]
        O[b] = (U[:D] / U[D:D + 1]).T
    return O



# revision 7
# speedup vs baseline: 1.1959x; 1.1959x over previous
"""Causal single-head attention (B=4, T=4096, C=1024, D=64) on 8 NeuronCores.

Sharding: core c = (batch b = c % 4, half h = c // 4).
Each core handles ALL queries of its batch, but only its half of the key
blocks (256-token key blocks with block index ≡ h mod 2).  Pure SPMD; cores
differ only in input data.  Each core emits unnormalized partial results
U^T = [V|1]^T @ exp(S^T) per query supertile; the host combines the two
halves per batch: O = (U0 + U1)[:64] / (U0 + U1)[64].

v2 layout/schedule (vs v1):
  * Projections col-packed: two C-chunks run concurrently in PE col groups
    (outputs land on PSUM partitions 0-63 / 64-127, DVE adds the halves).
  * Scores row-packed: qT/kT are duplicated across both partition halves so
    two 128-key tiles run concurrently in PE row groups (K=64 each).
  * Inputs streamed in 512-column blocks in consumption order; projections,
    scores/exp/PV software-pipelined in emission order so the ScalarE exp
    chain (~40us) paces the kernel and PE work hides underneath it.
"""
import sys
from collections import deque
import numpy as np
import ml_dtypes

if "/opt/trn_rl_repo" not in sys.path:
    sys.path.insert(0, "/opt/trn_rl_repo")

import concourse.bacc as bacc
import concourse.mybir as mybir
from concourse import tile
from concourse import bass_utils

bf16 = mybir.dt.bfloat16
f32 = mybir.dt.float32
BF = ml_dtypes.bfloat16

B, T, C, D = 4, 4096, 1024, 64
NST = 8          # query supertiles per batch (512 queries each)
STQ = 512
TK = T // 2      # key tokens per core
NKT = TK // 128  # local 128-key tiles per core (16)
NKB = TK // 512  # 512-key xk blocks (4)
NC_ = C // 128   # 8 c-tiles

_CACHE = {}


def _build():
    nc = bacc.Bacc(None, target_bir_lowering=False, debug=False, num_devices=8)

    xq = nc.dram_tensor("xq", [C, T], bf16, kind="ExternalInput")
    xk = nc.dram_tensor("xk", [C, TK], bf16, kind="ExternalInput")
    w = nc.dram_tensor("w", [C, 192], bf16, kind="ExternalInput")   # Wq|Wk|Wv
    msk = nc.dram_tensor("msk", [256, STQ], bf16, kind="ExternalInput")
    idn = nc.dram_tensor("idn", [65, 65], bf16, kind="ExternalInput")
    out = nc.dram_tensor("out", [65, T], f32, kind="ExternalOutput")

    with tile.TileContext(nc) as tc:
        with tc.tile_pool(name="sb", bufs=1) as sb, \
             tc.tile_pool(name="xqp", bufs=4) as xqp, \
             tc.tile_pool(name="xkp", bufs=2) as xkp, \
             tc.tile_pool(name="pp", bufs=3) as pp, \
             tc.tile_pool(name="usp", bufs=2) as usp, \
             tc.tile_pool(name="ps_s", bufs=2, space="PSUM") as ps_s, \
             tc.tile_pool(name="ps_u", bufs=2, space="PSUM") as ps_u, \
             tc.tile_pool(name="ps_a", bufs=2, space="PSUM") as ps_a:

            # ---- small resident inputs ----
            w_t = sb.tile([128, NC_ * 192], bf16, tag="w")
            msk_t = sb.tile([128, 2 * STQ], bf16, tag="msk")
            idn_t = sb.tile([65, 65], bf16, tag="idn")
            for c in range(NC_):
                nc.sync.dma_start(w_t[:, 192 * c:192 * (c + 1)],
                                  w[128 * c:128 * (c + 1), :])
            nc.sync.dma_start(msk_t[:, 0:STQ], msk[0:128, :])
            nc.sync.dma_start(msk_t[:, STQ:2 * STQ], msk[128:256, :])
            nc.sync.dma_start(idn_t[:], idn[:])

            # ---- persistent intermediates ----
            # qT2/kT2: transposed projections duplicated on both partition
            # halves so row-packed score matmuls can source row group 64-127.
            qT2 = sb.tile([128, T], bf16, tag="qT2")
            kT2 = sb.tile([128, TK], bf16, tag="kT2")
            vT = sb.tile([65, TK], bf16, tag="vT")   # row 64 = ones
            vP = sb.tile([128, NKT * 65], bf16, tag="vP")  # V tiles [key, d|1]
            nc.vector.memset(vT[64:65, :], 1.0)

            # ---- streamed inputs: [128, 512*c] blocks, c-tiles side by side
            xqb = {}
            xkb = {}

            def dma_xq(st):
                t_ = xqp.tile([128, NC_ * 512], bf16, tag="xqb",
                              name=f"xqb{st}")
                xqb[st] = t_
                for c in range(NC_):
                    nc.sync.dma_start(
                        t_[:, 512 * c:512 * (c + 1)],
                        xq[128 * c:128 * (c + 1), STQ * st:STQ * (st + 1)])

            def dma_xk(b):
                t_ = xkp.tile([128, NC_ * 512], bf16, tag="xkb",
                              name=f"xkb{b}")
                xkb[b] = t_
                for c in range(NC_):
                    nc.sync.dma_start(
                        t_[:, 512 * c:512 * (c + 1)],
                        xk[128 * c:128 * (c + 1), 512 * b:512 * (b + 1)])

            # ---- projection emitters ----
            # Col-packed: the two PE col groups compute two independent
            # 64-row outputs concurrently (out partitions 0-63 / 64-127),
            # each accumulating over all 8 C-chunks — no cross-half adds.
            def q_proj2_items(stp):
                """Q^T for supertiles 2*stp (col group 0) and 2*stp+1
                (col group 1): same Wq weights, different moving operand."""
                st0, st1 = 2 * stp, 2 * stp + 1
                acc = ps_a.tile([128, STQ], f32, tag="acc",
                                name=f"qacc{stp}")
                items = []
                for c in range(NC_):
                    def mm(c=c, acc=acc):
                        for hh, st in ((0, st0), (1, st1)):
                            nc.tensor.matmul(
                                acc[64 * hh:64 * (hh + 1), :],
                                w_t[:, 192 * c:192 * c + 64],
                                xqb[st][:, 512 * c:512 * (c + 1)],
                                start=(c == 0), stop=(c == NC_ - 1))
                    items.append(mm)

                def fin(acc=acc):
                    for hh, st in ((0, st0), (1, st1)):
                        qsl = slice(STQ * st, STQ * (st + 1))
                        src = acc[64 * hh:64 * (hh + 1), :]
                        nc.vector.tensor_copy(qT2[0:64, qsl], src)
                        nc.vector.tensor_copy(qT2[64:128, qsl], src)
                items.append(fin)
                return items

            def kv_proj2_items(b):
                """K^T (col group 0) and V^T (col group 1) for xk block b:
                different weights, same moving operand."""
                ksl = slice(512 * b, 512 * (b + 1))
                acc = ps_a.tile([128, STQ], f32, tag="acc", name=f"kvacc{b}")
                items = []
                for c in range(NC_):
                    def mm(c=c, acc=acc):
                        for hh, wofs in ((0, 64), (1, 128)):
                            nc.tensor.matmul(
                                acc[64 * hh:64 * (hh + 1), :],
                                w_t[:, 192 * c + wofs:192 * c + wofs + 64],
                                xkb[b][:, 512 * c:512 * (c + 1)],
                                start=(c == 0), stop=(c == NC_ - 1))
                    items.append(mm)

                def fin(acc=acc):
                    nc.vector.tensor_copy(kT2[0:64, ksl], acc[0:64, :])
                    nc.vector.tensor_copy(kT2[64:128, ksl], acc[0:64, :])
                    nc.vector.tensor_copy(vT[0:64, ksl], acc[64:128, :])
                items.append(fin)

                for j in range(4 * b, 4 * b + 4):
                    def tr(j=j):
                        tp = ps_a.tile([128, 65], bf16, tag="acc",
                                       name=f"tp{j}")
                        nc.tensor.transpose(tp[:],
                                            vT[:, 128 * j:128 * (j + 1)],
                                            idn_t[:])
                        nc.vector.tensor_copy(vP[:, 65 * j:65 * (j + 1)],
                                              tp[:])
                    items.append(tr)
                return items

            # ---- attention pair stream ----
            pairs = [(st, k) for st in range(NST) for k in range(st + 1)]
            s2_of = {}
            p2_of = {}
            u_of = {}

            def scores(i):
                st, k = pairs[i]
                qsl = slice(STQ * st, STQ * (st + 1))
                s2 = ps_s.tile([128, 2 * STQ], f32, tag="s", name=f"s{i}")
                s2_of[i] = s2
                j0, j1 = 2 * k, 2 * k + 1
                nc.tensor.matmul(s2[:, 0:STQ],
                                 kT2[0:64, 128 * j0:128 * (j0 + 1)],
                                 qT2[0:64, qsl], start=True, stop=True)
                nc.tensor.matmul(s2[:, STQ:2 * STQ],
                                 kT2[64:128, 128 * j1:128 * (j1 + 1)],
                                 qT2[64:128, qsl], start=True, stop=True)

            def exp_mask(i):
                st, k = pairs[i]
                p2 = pp.tile([128, 2 * STQ], bf16, tag="p", name=f"p{i}")
                p2_of[i] = p2
                nc.scalar.activation(p2[:], s2_of[i][:],
                                     mybir.ActivationFunctionType.Exp,
                                     scale=0.125)
                if k == st:   # diagonal pair -> causal masks
                    nc.vector.tensor_mul(p2[:], p2[:], msk_t[:])
                del s2_of[i]

            def pv(i):
                st, k = pairs[i]
                if k == 0:
                    u_of[st] = ps_u.tile([65, STQ], f32, tag="u",
                                         name=f"u{st}")
                u = u_of[st]
                p2 = p2_of.pop(i)
                for dd in range(2):
                    j = 2 * k + dd
                    nc.tensor.matmul(u[:], vP[:, 65 * j:65 * (j + 1)],
                                     p2[:, STQ * dd:STQ * (dd + 1)],
                                     start=(j == 0), stop=(j == 2 * st + 1))

            def drain_u(st):
                qsl = slice(STQ * st, STQ * (st + 1))
                u_sb = usp.tile([65, STQ], f32, tag="usb", name=f"usb{st}")
                nc.vector.tensor_copy(u_sb[:], u_of.pop(st)[:])
                nc.sync.dma_start(out[:, qsl], u_sb[:])

            # ---- emission schedule ----
            bg = deque()

            # preamble: first blocks + their projections, next DMAs queued
            dma_xk(0)
            dma_xq(0)
            dma_xq(1)
            for it in kv_proj2_items(0):
                it()
            for it in q_proj2_items(0):
                it()
            dma_xk(1)
            dma_xq(2)
            dma_xq(3)

            dma_plan = {0: [lambda: dma_xk(2)],
                        1: [lambda: dma_xq(4), lambda: dma_xq(5)],
                        2: [lambda: dma_xk(3)],
                        3: [lambda: dma_xq(6), lambda: dma_xq(7)]}
            # (enqueue_st, deadline_st): projections must be fully EMITTED
            # before any score of deadline_st is emitted (Tile deps follow
            # trace order — a consumer traced before its producer reads
            # stale data).
            work_plan = {0: (lambda: kv_proj2_items(1), 2),
                         1: (lambda: q_proj2_items(1), 2),
                         2: (lambda: kv_proj2_items(2), 4),
                         3: (lambda: q_proj2_items(2), 4),
                         4: (lambda: kv_proj2_items(3), 6),
                         5: (lambda: q_proj2_items(3), 6)}

            scores(0)
            for i, (st, k) in enumerate(pairs):
                if k == 0:
                    for d_ in dma_plan.get(st, []):
                        d_()
                    if st in work_plan:
                        gen, dl = work_plan[st]
                        bg.extend((dl, it) for it in gen())
                if i + 1 < len(pairs):
                    nst = pairs[i + 1][0]
                    if nst != st:   # crossing a supertile boundary
                        while bg and bg[0][0] <= nst:
                            bg.popleft()[1]()
                    scores(i + 1)
                exp_mask(i)
                pv(i)
                if k == st:
                    drain_u(st)
                # keep PE fed just below the ~1.1us ScalarE exp per pair
                budget = 3 if st < 3 else 2
                for _ in range(budget):
                    if bg:
                        bg.popleft()[1]()
            while bg:
                bg.popleft()[1]()

    nc.compile()
    return nc


def _get_nc():
    if "nc" not in _CACHE:
        _CACHE["nc"] = _build()
    return _CACHE["nc"]


def kernel(x, Wq, Wk, Wv, _trace=False):
    x = np.asarray(x)
    nc = _get_nc()

    xT = np.ascontiguousarray(x.transpose(0, 2, 1)).astype(BF)   # [B, C, T]
    w = np.concatenate([Wq, Wk, Wv], axis=1).astype(BF)          # [C, 192]
    idn = np.eye(65, dtype=BF)

    j = np.arange(128)[:, None]
    i = np.arange(STQ)[None, :]
    masks = {}
    for h in range(2):
        m0 = (j <= i - 256 * h).astype(BF)
        m1 = (j <= i - 256 * h - 128).astype(BF)
        masks[h] = np.concatenate([m0, m1], axis=0)

    # key-token selector: 256-blocks with block index ≡ h (mod 2)
    tok = np.arange(T)
    keysel = {h: ((tok // 256) % 2 == h) for h in range(2)}

    in_maps = []
    for c in range(8):
        b, h = c % 4, c // 4
        in_maps.append({
            "xq": xT[b],
            "xk": np.ascontiguousarray(xT[b][:, keysel[h]]),
            "w": w,
            "msk": masks[h],
            "idn": idn,
        })

    res = bass_utils.run_bass_kernel_spmd(nc, in_maps, core_ids=list(range(8)),
                                          trace=_trace)
    _CACHE["last_results"] = res

    O = np.empty((B, T, D), dtype=np.float32)
    for b in range(B):
        U = res.results[b]["out"] + res.results[b + 4]["out"]    # [65, T]
        O[b] = (U[:D] / U[D:D + 1]).T
    return O


# revision 15
# speedup vs baseline: 1.3902x; 1.1625x over previous
"""Causal single-head attention (B=4, T=4096, C=1024, D=64) on 8 NeuronCores.

Sharding: core c = (batch b = c % 4, half h = c // 4).
Each core handles ALL queries of its batch, but only its half of the key
blocks (256-token key blocks with block index ≡ h mod 2).  Pure SPMD; cores
differ only in input data.  Each core emits unnormalized partial results
U^T = [V|1]^T @ exp(S^T) per query supertile; the host combines the two
halves per batch: O = (U0 + U1)[:64] / (U0 + U1)[64].

v3 I/O + schedule (vs v1):
  * Single streamed input: x^T pre-tiled on host into 8 supertile blocks
    [C, 512], each column-rotated by 256*h so THIS core's key columns are
    always the first 256 of every 512-column chunk.  K/V projections slice
    keys straight out of the query stream - no separate xk input.  Masks
    and outputs are correspondingly permuted host-side.
  * Projections col-packed: two 64-row outputs run concurrently in the two
    PE col groups (Q of two supertiles; K and V of one block).
  * Scores row-packed: qT/kT duplicated on both partition halves so two
    128-key tiles run concurrently in PE row groups (K=64 each).
  * Everything software-pipelined in emission order; the ScalarE exp chain
    (36 x ~1.1us) paces the kernel, PE/DMA/DVE hide underneath.
"""
import sys
from collections import deque
import numpy as np
import ml_dtypes

if "/opt/trn_rl_repo" not in sys.path:
    sys.path.insert(0, "/opt/trn_rl_repo")

import concourse.bacc as bacc
import concourse.mybir as mybir
from concourse import tile
from concourse import bass_utils

bf16 = mybir.dt.bfloat16
f32 = mybir.dt.float32
BF = ml_dtypes.bfloat16

B, T, C, D = 4, 4096, 1024, 64
NST = 8          # query supertiles per batch (512 queries each)
STQ = 512
TK = T // 2      # key tokens per core
NKT = TK // 128  # local 128-key tiles per core (16)
NKB = TK // 512  # local 512-key blocks per core (4)
NC_ = C // 128   # 8 c-tiles

_CACHE = {}


def _build():
    nc = bacc.Bacc(None, target_bir_lowering=False, debug=False, num_devices=8)

    # x^T pre-tiled to SBUF layout: row 128*st + p holds, for partition p,
    # the (c-chunk, column) free dim of supertile st; columns of each
    # supertile are per-core rotated so cols [0:256) of every 512-column
    # chunk are THIS core's key tokens.
    xq = nc.dram_tensor("xq", [NST * 128, NC_ * STQ], bf16,
                        kind="ExternalInput")
    w = nc.dram_tensor("w", [128, NC_ * 192], bf16,
                       kind="ExternalInput")   # Wq|Wk|Wv, c-chunks packed
    msk = nc.dram_tensor("msk", [256, STQ], bf16, kind="ExternalInput")
    idn = nc.dram_tensor("idn", [65, 65], bf16, kind="ExternalInput")
    out = nc.dram_tensor("out", [65, T], f32, kind="ExternalOutput")

    with tile.TileContext(nc) as tc:
        with tc.tile_pool(name="sb", bufs=1) as sb, \
             tc.tile_pool(name="xqp", bufs=4) as xqp, \
             tc.tile_pool(name="pp", bufs=3) as pp, \
             tc.tile_pool(name="usp", bufs=2) as usp, \
             tc.tile_pool(name="ps_s", bufs=2, space="PSUM") as ps_s, \
             tc.tile_pool(name="ps_u", bufs=2, space="PSUM") as ps_u, \
             tc.tile_pool(name="ps_a", bufs=2, space="PSUM") as ps_a:

            # ---- small resident inputs ----
            w_t = sb.tile([128, NC_ * 192], bf16, tag="w")
            msk_t = sb.tile([128, 2 * STQ], bf16, tag="msk")
            idn_t = sb.tile([65, 65], bf16, tag="idn")
            nc.sync.dma_start(w_t[:], w[:])
            nc.sync.dma_start(msk_t[:, 0:STQ], msk[0:128, :])
            nc.sync.dma_start(msk_t[:, STQ:2 * STQ], msk[128:256, :])
            nc.sync.dma_start(idn_t[:], idn[:])

            # ---- persistent intermediates ----
            # qT2/kT2: transposed projections duplicated on both partition
            # halves so row-packed score matmuls can source row group 64-127.
            qT2 = sb.tile([128, T], bf16, tag="qT2")
            kT2 = sb.tile([128, TK], bf16, tag="kT2")
            vT = sb.tile([65, TK], bf16, tag="vT")   # row 64 = ones
            vP = sb.tile([128, NKT * 65], bf16, tag="vP")  # V tiles [key, d|1]
            nc.vector.memset(vT[64:65, :], 1.0)

            # ---- streamed input blocks: [128, (c, col)] layout ----
            xqb = {}

            def dma_xq(st):
                t_ = xqp.tile([128, NC_ * STQ], bf16, tag="xqb",
                              name=f"xqb{st}")
                xqb[st] = t_
                for piece in range(2):   # c-chunks 0-3 / 4-7
                    csl = slice(2048 * piece, 2048 * (piece + 1))
                    nc.sync.dma_start(
                        t_[:, csl],
                        xq[128 * st:128 * (st + 1), csl])

            # ---- projection emitters ----
            # Col-packed: the two PE col groups compute two independent
            # 64-row outputs concurrently (out partitions 0-63 / 64-127),
            # each accumulating over all 8 C-chunks.
            def q_proj2_items(stp):
                """Q^T for supertiles 2*stp (col group 0) and 2*stp+1
                (col group 1): same Wq weights, different moving operand."""
                st0, st1 = 2 * stp, 2 * stp + 1
                acc = ps_a.tile([128, STQ], f32, tag="acc",
                                name=f"qacc{stp}")
                items = []
                for c in range(NC_):
                    def mm(c=c, acc=acc):
                        for hh, st in ((0, st0), (1, st1)):
                            nc.tensor.matmul(
                                acc[64 * hh:64 * (hh + 1), :],
                                w_t[:, 192 * c:192 * c + 64],
                                xqb[st][:, 512 * c:512 * (c + 1)],
                                start=(c == 0), stop=(c == NC_ - 1))
                    items.append(mm)

                def fin(acc=acc):
                    for hh, st in ((0, st0), (1, st1)):
                        qsl = slice(STQ * st, STQ * (st + 1))
                        src = acc[64 * hh:64 * (hh + 1), :]
                        nc.vector.tensor_copy(qT2[0:64, qsl], src)
                        nc.vector.tensor_copy(qT2[64:128, qsl], src)
                items.append(fin)
                return items

            def kv_proj2_items(b):
                """K^T and V^T for local key block b = the leading
                256-column key halves of supertile blocks 2b and 2b+1.
                Col-packed per tensor: piece 0 (keys 512b+[0,256)) in col
                group 0, piece 1 in col group 1 (partition-split groups
                have well-defined per-half PSUM accumulation)."""
                accK = ps_a.tile([128, 256], f32, tag="acc", name=f"kacc{b}")
                accV = ps_a.tile([128, 256], f32, tag="acc", name=f"vacc{b}")
                items = []
                for c in range(NC_):
                    def mm(c=c, accK=accK, accV=accV):
                        for hh in range(2):          # source block 2b+hh
                            src = xqb[2 * b + hh]
                            for acc, wofs in ((accK, 64), (accV, 128)):
                                nc.tensor.matmul(
                                    acc[64 * hh:64 * (hh + 1), :],
                                    w_t[:, 192 * c + wofs:192 * c + wofs + 64],
                                    src[:, 512 * c:512 * c + 256],
                                    start=(c == 0), stop=(c == NC_ - 1))
                    items.append(mm)

                def fin(accK=accK, accV=accV):
                    for hh in range(2):
                        ksl = slice(512 * b + 256 * hh,
                                    512 * b + 256 * (hh + 1))
                        src = accK[64 * hh:64 * (hh + 1), :]
                        nc.vector.tensor_copy(kT2[0:64, ksl], src)
                        nc.vector.tensor_copy(kT2[64:128, ksl], src)
                        nc.vector.tensor_copy(
                            vT[0:64, ksl], accV[64 * hh:64 * (hh + 1), :])
                items.append(fin)

                for j in range(4 * b, 4 * b + 4):
                    def tr(j=j):
                        tp = ps_a.tile([128, 65], bf16, tag="acc",
                                       name=f"tp{j}")
                        nc.tensor.transpose(tp[:],
                                            vT[:, 128 * j:128 * (j + 1)],
                                            idn_t[:])
                        nc.vector.tensor_copy(vP[:, 65 * j:65 * (j + 1)],
                                              tp[:])
                    items.append(tr)
                return items

            # ---- attention pair stream ----
            pairs = [(st, k) for st in range(NST) for k in range(st + 1)]
            s2_of = {}
            p2_of = {}
            u_of = {}

            def scores(i):
                st, k = pairs[i]
                qsl = slice(STQ * st, STQ * (st + 1))
                s2 = ps_s.tile([128, 2 * STQ], f32, tag="s", name=f"s{i}")
                s2_of[i] = s2
                j0, j1 = 2 * k, 2 * k + 1
                nc.tensor.matmul(s2[:, 0:STQ],
                                 kT2[0:64, 128 * j0:128 * (j0 + 1)],
                                 qT2[0:64, qsl], start=True, stop=True)
                nc.tensor.matmul(s2[:, STQ:2 * STQ],
                                 kT2[64:128, 128 * j1:128 * (j1 + 1)],
                                 qT2[64:128, qsl], start=True, stop=True)

            def exp_mask(i):
                st, k = pairs[i]
                p2 = pp.tile([128, 2 * STQ], bf16, tag="p", name=f"p{i}")
                p2_of[i] = p2
                nc.scalar.activation(p2[:], s2_of[i][:],
                                     mybir.ActivationFunctionType.Exp,
                                     scale=0.125)
                if k == st:   # diagonal pair -> causal masks
                    nc.vector.tensor_mul(p2[:], p2[:], msk_t[:])
                del s2_of[i]

            def pv(i):
                st, k = pairs[i]
                if k == 0:
                    u_of[st] = ps_u.tile([65, STQ], f32, tag="u",
                                         name=f"u{st}")
                u = u_of[st]
                p2 = p2_of.pop(i)
                for dd in range(2):
                    j = 2 * k + dd
                    nc.tensor.matmul(u[:], vP[:, 65 * j:65 * (j + 1)],
                                     p2[:, STQ * dd:STQ * (dd + 1)],
                                     start=(j == 0), stop=(j == 2 * st + 1))

            def drain_u(st):
                qsl = slice(STQ * st, STQ * (st + 1))
                u_sb = usp.tile([65, STQ], f32, tag="usb", name=f"usb{st}")
                nc.vector.tensor_copy(u_sb[:], u_of.pop(st)[:])
                nc.sync.dma_start(out[:, qsl], u_sb[:])

            # ---- emission schedule ----
            bg = deque()

            dma_xq(0)
            dma_xq(1)
            for it in kv_proj2_items(0):
                it()
            for it in q_proj2_items(0):
                it()
            dma_xq(2)
            dma_xq(3)

            dma_plan = {1: [lambda: dma_xq(4), lambda: dma_xq(5)],
                        3: [lambda: dma_xq(6), lambda: dma_xq(7)]}
            # (generator, deadline_st): items must be fully EMITTED before
            # any score of deadline_st is emitted (Tile deps follow trace
            # order - a consumer traced before its producer reads garbage).
            work_plan = {0: (lambda: kv_proj2_items(1), 2),
                         1: (lambda: q_proj2_items(1), 2),
                         2: (lambda: kv_proj2_items(2), 4),
                         3: (lambda: q_proj2_items(2), 4),
                         4: (lambda: kv_proj2_items(3), 6),
                         5: (lambda: q_proj2_items(3), 6)}

            scores(0)
            for i, (st, k) in enumerate(pairs):
                if k == 0:
                    for d_ in dma_plan.get(st, []):
                        d_()
                    if st in work_plan:
                        gen, dl = work_plan[st]
                        bg.extend((dl, it) for it in gen())
                if i + 1 < len(pairs):
                    nst = pairs[i + 1][0]
                    if nst != st:   # crossing a supertile boundary
                        while bg and bg[0][0] <= nst:
                            bg.popleft()[1]()
                    scores(i + 1)
                exp_mask(i)
                pv(i)
                if k == st:
                    drain_u(st)
                # keep PE fed just below the ~1.1us ScalarE exp per pair
                budget = 3 if st < 3 else 2
                for _ in range(budget):
                    if bg:
                        bg.popleft()[1]()
            while bg:
                bg.popleft()[1]()

    nc.compile()
    return nc


def _get_nc():
    if "nc" not in _CACHE:
        _CACHE["nc"] = _build()
    return _CACHE["nc"]


def kernel(x, Wq, Wk, Wv, _trace=False):
    x = np.asarray(x)
    nc = _get_nc()

    xT = np.ascontiguousarray(x.transpose(0, 2, 1)).astype(BF)   # [B, C, T]
    w = np.concatenate([Wq, Wk, Wv], axis=1).astype(BF)          # [C, 192]
    # pack to device layout [128, (c, k)]: row p, col 192*c+k = w[128c+p, k]
    w2 = np.ascontiguousarray(
        w.reshape(NC_, 128, 192).transpose(1, 0, 2)).reshape(128, NC_ * 192)
    idn = np.eye(65, dtype=BF)

    # Column-rotated supertile blocks in device SBUF layout
    # [st*128 + p, 512*c + j]: core (b, h) sees supertile st with columns
    # rolled left by 256*h, so its key half is always cols [0:256).
    xqs = {}
    for bidx in range(B):
        blocks = xT[bidx].reshape(C, NST, STQ).transpose(1, 0, 2)  # [st,C,q]
        for h in range(2):
            rb = np.roll(blocks, -256 * h, axis=2) if h else blocks
            # [st, C, q] -> [st, p, c, q] -> [st*128, c*512]
            xqs[(bidx, h)] = np.ascontiguousarray(
                rb.reshape(NST, NC_, 128, STQ).transpose(0, 2, 1, 3)
            ).reshape(NST * 128, NC_ * STQ)

    # Masks in permuted query coordinates: query column j of a supertile is
    # global offset (j + 256h) % 512; diag tile d covers keys 256h+128d+r.
    jj = np.arange(STQ)[None, :]
    rr = np.arange(128)[:, None]
    masks = {}
    for h in range(2):
        gq = (jj + 256 * h) % 512
        m0 = (rr <= gq - 256 * h).astype(BF)
        m1 = (rr <= gq - 256 * h - 128).astype(BF)
        masks[h] = np.concatenate([m0, m1], axis=0)

    in_maps = []
    for cid in range(8):
        bidx, h = cid % 4, cid // 4
        in_maps.append({
            "xq": xqs[(bidx, h)],
            "w": w2,
            "msk": masks[h],
            "idn": idn,
        })

    res = bass_utils.run_bass_kernel_spmd(nc, in_maps, core_ids=list(range(8)),
                                          trace=_trace)
    _CACHE["last_results"] = res

    O = np.empty((B, T, D), dtype=np.float32)
    for bidx in range(B):
        U = np.zeros((65, T), dtype=np.float32)
        for h in range(2):
            part = res.results[bidx + 4 * h]["out"]        # [65, T] permuted
            blocks = part.reshape(65, NST, STQ)
            U += np.roll(blocks, 256 * h, axis=2).reshape(65, T)
        O[bidx] = (U[:D] / U[D:D + 1]).T
    return O


# revision 18
# speedup vs baseline: 1.4346x; 1.0319x over previous
"""Causal single-head attention (B=4, T=4096, C=1024, D=64) on 8 NeuronCores.

Sharding: core c = (batch b = c % 4, half h = c // 4).
Each core handles ALL queries of its batch, but only its half of the key
blocks (256-token key blocks with block index ≡ h mod 2).  Pure SPMD; cores
differ only in input data.  Each core emits unnormalized partial results
U^T = [V|1]^T @ exp(S^T) per query supertile; the host combines the two
halves per batch: O = (U0 + U1)[:64] / (U0 + U1)[64].

v3 I/O + schedule (vs v1):
  * Single streamed input: x^T pre-tiled on host into 8 supertile blocks
    [C, 512], each column-rotated by 256*h so THIS core's key columns are
    always the first 256 of every 512-column chunk.  K/V projections slice
    keys straight out of the query stream - no separate xk input.  Masks
    and outputs are correspondingly permuted host-side.
  * Projections col-packed: two 64-row outputs run concurrently in the two
    PE col groups (Q of two supertiles; K and V of one block).
  * Scores row-packed: qT/kT duplicated on both partition halves so two
    128-key tiles run concurrently in PE row groups (K=64 each).
  * Everything software-pipelined in emission order; the ScalarE exp chain
    (36 x ~1.1us) paces the kernel, PE/DMA/DVE hide underneath.
"""
import sys
from collections import deque
import numpy as np
import ml_dtypes

if "/opt/trn_rl_repo" not in sys.path:
    sys.path.insert(0, "/opt/trn_rl_repo")

import concourse.bacc as bacc
import concourse.mybir as mybir
from concourse import tile
from concourse import bass_utils

bf16 = mybir.dt.bfloat16
f32 = mybir.dt.float32
BF = ml_dtypes.bfloat16

B, T, C, D = 4, 4096, 1024, 64
NST = 8          # query supertiles per batch (512 queries each)
STQ = 512
TK = T // 2      # key tokens per core
NKT = TK // 128  # local 128-key tiles per core (16)
NKB = TK // 512  # local 512-key blocks per core (4)
NC_ = C // 128   # 8 c-tiles

_CACHE = {}


def _build():
    nc = bacc.Bacc(None, target_bir_lowering=False, debug=False, num_devices=8)

    # x^T pre-tiled to SBUF layout: row 128*st + p holds, for partition p,
    # the (c-chunk, column) free dim of supertile st; columns of each
    # supertile are per-core rotated so cols [0:256) of every 512-column
    # chunk are THIS core's key tokens.
    xq = nc.dram_tensor("xq", [NST * 128, NC_ * STQ], bf16,
                        kind="ExternalInput")
    w = nc.dram_tensor("w", [128, NC_ * 192], bf16,
                       kind="ExternalInput")   # Wq|Wk|Wv, c-chunks packed
    msk = nc.dram_tensor("msk", [256, STQ], bf16, kind="ExternalInput")
    idn = nc.dram_tensor("idn", [65, 65], bf16, kind="ExternalInput")
    out = nc.dram_tensor("out", [65, T], f32, kind="ExternalOutput")

    with tile.TileContext(nc) as tc:
        with tc.tile_pool(name="sb", bufs=1) as sb, \
             tc.tile_pool(name="xqp", bufs=4) as xqp, \
             tc.tile_pool(name="pp", bufs=3) as pp, \
             tc.tile_pool(name="usp", bufs=2) as usp, \
             tc.tile_pool(name="ps_s", bufs=2, space="PSUM") as ps_s, \
             tc.tile_pool(name="ps_u", bufs=2, space="PSUM") as ps_u, \
             tc.tile_pool(name="ps_a", bufs=2, space="PSUM") as ps_a:

            # ---- small resident inputs ----
            w_t = sb.tile([128, NC_ * 192], bf16, tag="w")
            msk_t = sb.tile([128, 2 * STQ], bf16, tag="msk")
            idn_t = sb.tile([65, 65], bf16, tag="idn")
            nc.sync.dma_start(w_t[:], w[:])
            nc.sync.dma_start(msk_t[:, 0:STQ], msk[0:128, :])
            nc.sync.dma_start(msk_t[:, STQ:2 * STQ], msk[128:256, :])
            nc.sync.dma_start(idn_t[:], idn[:])

            # ---- persistent intermediates ----
            # qT2/kT2: transposed projections duplicated on both partition
            # halves so row-packed score matmuls can source row group 64-127.
            qT2 = sb.tile([128, T], bf16, tag="qT2")
            kT2 = sb.tile([128, TK], bf16, tag="kT2")
            vT = sb.tile([65, TK], bf16, tag="vT")   # row 64 = ones
            vP = sb.tile([128, NKT * 65], bf16, tag="vP")  # V tiles [key, d|1]
            nc.vector.memset(vT[64:65, :], 1.0)

            # ---- streamed input blocks: [128, (c, col)] layout ----
            xqb = {}

            def dma_xq(st, pieces=2):
                t_ = xqp.tile([128, NC_ * STQ], bf16, tag="xqb",
                              name=f"xqb{st}")
                xqb[st] = t_
                npc = NC_ * STQ // pieces
                for piece in range(pieces):
                    csl = slice(npc * piece, npc * (piece + 1))
                    nc.sync.dma_start(
                        t_[:, csl],
                        xq[128 * st:128 * (st + 1), csl])

            # ---- projection emitters ----
            # Col-packed: the two PE col groups compute two independent
            # 64-row outputs concurrently (out partitions 0-63 / 64-127),
            # each accumulating over all 8 C-chunks.
            def q_proj2_items(stp):
                """Q^T for supertiles 2*stp (col group 0) and 2*stp+1
                (col group 1): same Wq weights, different moving operand."""
                st0, st1 = 2 * stp, 2 * stp + 1
                acc = ps_a.tile([128, STQ], f32, tag="acc",
                                name=f"qacc{stp}")
                items = []
                for c in range(NC_):
                    def mm(c=c, acc=acc):
                        for hh, st in ((0, st0), (1, st1)):
                            nc.tensor.matmul(
                                acc[64 * hh:64 * (hh + 1), :],
                                w_t[:, 192 * c:192 * c + 64],
                                xqb[st][:, 512 * c:512 * (c + 1)],
                                start=(c == 0), stop=(c == NC_ - 1))
                    items.append(mm)

                def fin(acc=acc):
                    for hh, st in ((0, st0), (1, st1)):
                        qsl = slice(STQ * st, STQ * (st + 1))
                        src = acc[64 * hh:64 * (hh + 1), :]
                        nc.vector.tensor_copy(qT2[0:64, qsl], src)
                        nc.vector.tensor_copy(qT2[64:128, qsl], src)
                items.append(fin)
                return items

            def kv_proj2_items(b):
                """K^T and V^T for local key block b = the leading
                256-column key halves of supertile blocks 2b and 2b+1.
                Col-packed per tensor: piece 0 (keys 512b+[0,256)) in col
                group 0, piece 1 in col group 1 (partition-split groups
                have well-defined per-half PSUM accumulation)."""
                def pidx(st, k):
                    return st * (st + 1) // 2 + k

                dl = pidx(2 * b, 2 * b)          # first read: diag pair
                accK = ps_a.tile([128, 256], f32, tag="acc", name=f"kacc{b}")
                accV = ps_a.tile([128, 256], f32, tag="acc", name=f"vacc{b}")
                items = []
                for c in range(NC_):
                    def mm(c=c, accK=accK, accV=accV):
                        for hh in range(2):          # source block 2b+hh
                            src = xqb[2 * b + hh]
                            for acc, wofs in ((accK, 64), (accV, 128)):
                                nc.tensor.matmul(
                                    acc[64 * hh:64 * (hh + 1), :],
                                    w_t[:, 192 * c + wofs:192 * c + wofs + 64],
                                    src[:, 512 * c:512 * c + 256],
                                    start=(c == 0), stop=(c == NC_ - 1))
                    items.append((dl, mm))

                def fin(accK=accK, accV=accV):
                    for hh in range(2):
                        ksl = slice(512 * b + 256 * hh,
                                    512 * b + 256 * (hh + 1))
                        src = accK[64 * hh:64 * (hh + 1), :]
                        nc.vector.tensor_copy(kT2[0:64, ksl], src)
                        nc.vector.tensor_copy(kT2[64:128, ksl], src)
                        nc.vector.tensor_copy(
                            vT[0:64, ksl], accV[64 * hh:64 * (hh + 1), :])
                items.append((dl, fin))

                for j in range(4 * b, 4 * b + 4):
                    def tr(j=j):
                        tp = ps_a.tile([128, 65], bf16, tag="acc",
                                       name=f"tp{j}")
                        nc.tensor.transpose(tp[:],
                                            vT[:, 128 * j:128 * (j + 1)],
                                            idn_t[:])
                        nc.vector.tensor_copy(vP[:, 65 * j:65 * (j + 1)],
                                              tp[:])
                    jst = 2 * b + (j - 4 * b) // 2   # diag st using tile j
                    items.append((pidx(jst, jst), tr))
                return items

            # ---- attention pair stream ----
            pairs = [(st, k) for st in range(NST) for k in range(st + 1)]
            s2_of = {}
            p2_of = {}
            u_of = {}

            def scores(i):
                st, k = pairs[i]
                qsl = slice(STQ * st, STQ * (st + 1))
                s2 = ps_s.tile([128, 2 * STQ], f32, tag="s", name=f"s{i}")
                s2_of[i] = s2
                j0, j1 = 2 * k, 2 * k + 1
                nc.tensor.matmul(s2[:, 0:STQ],
                                 kT2[0:64, 128 * j0:128 * (j0 + 1)],
                                 qT2[0:64, qsl], start=True, stop=True)
                nc.tensor.matmul(s2[:, STQ:2 * STQ],
                                 kT2[64:128, 128 * j1:128 * (j1 + 1)],
                                 qT2[64:128, qsl], start=True, stop=True)

            def exp_mask(i):
                st, k = pairs[i]
                p2 = pp.tile([128, 2 * STQ], bf16, tag="p", name=f"p{i}")
                p2_of[i] = p2
                nc.scalar.activation(p2[:], s2_of[i][:],
                                     mybir.ActivationFunctionType.Exp,
                                     scale=0.125)
                if k == st:   # diagonal pair -> causal masks
                    nc.vector.tensor_mul(p2[:], p2[:], msk_t[:])
                del s2_of[i]

            def pv(i):
                st, k = pairs[i]
                if k == 0:
                    u_of[st] = ps_u.tile([65, STQ], f32, tag="u",
                                         name=f"u{st}")
                u = u_of[st]
                p2 = p2_of.pop(i)
                for dd in range(2):
                    j = 2 * k + dd
                    nc.tensor.matmul(u[:], vP[:, 65 * j:65 * (j + 1)],
                                     p2[:, STQ * dd:STQ * (dd + 1)],
                                     start=(j == 0), stop=(j == 2 * st + 1))

            def drain_u(st):
                qsl = slice(STQ * st, STQ * (st + 1))
                u_sb = usp.tile([65, STQ], f32, tag="usb", name=f"usb{st}")
                nc.vector.tensor_copy(u_sb[:], u_of.pop(st)[:])
                nc.sync.dma_start(out[:, qsl], u_sb[:])

            # ---- emission schedule ----
            def P(st, k):   # global pair index
                return st * (st + 1) // 2 + k

            bg = deque()

            # HAM warm-up: a dense stream of cheap matmuls from ~t=2us keeps
            # the PE clock-gate busy through the DMA ramp so the real
            # preamble runs at 2.4 GHz instead of 1.2.
            scr_w = sb.tile([128, 32], bf16, tag="scrw")
            scr_r = sb.tile([128, 128], bf16, tag="scrr")
            nc.vector.memset(scr_w[:], 0.0)
            nc.vector.memset(scr_r[:], 0.0)
            dmy = ps_a.tile([32, 128], f32, tag="acc", name="dmy")
            dma_xq(0, pieces=4)
            dma_xq(1, pieces=4)
            for _ in range(96):
                nc.tensor.matmul(dmy[:], scr_w[:], scr_r[:],
                                 start=True, stop=True)

            for _, it in kv_proj2_items(0):
                it()
            for it in q_proj2_items(0):
                it()
            dma_xq(2)
            dma_xq(3)

            dma_plan = {1: [lambda: dma_xq(4), lambda: dma_xq(5)],
                        3: [lambda: dma_xq(6), lambda: dma_xq(7)]}
            # st -> generator of (deadline_pair_idx, item): items must be
            # fully EMITTED before the score of that pair is emitted (Tile
            # deps follow trace order - a consumer traced before its
            # producer reads garbage).  Q feeds the first pair of its
            # supertile; K/V tiles are first read by their diagonal pair.
            work_plan = {
                0: lambda: [(P(2, 0), it) for it in q_proj2_items(1)],
                1: lambda: kv_proj2_items(1),
                2: lambda: [(P(4, 0), it) for it in q_proj2_items(2)],
                3: lambda: kv_proj2_items(2),
                4: lambda: [(P(6, 0), it) for it in q_proj2_items(3)],
                5: lambda: kv_proj2_items(3),
            }

            scores(0)
            for i, (st, k) in enumerate(pairs):
                if k == 0:
                    for d_ in dma_plan.get(st, []):
                        d_()
                    if st in work_plan:
                        bg.extend(work_plan[st]())
                if i + 1 < len(pairs):
                    while bg and bg[0][0] <= i + 1:   # due before next pair
                        bg.popleft()[1]()
                    scores(i + 1)
                exp_mask(i)
                pv(i)
                if k == st:
                    drain_u(st)
                # keep PE fed just below the ~1.1us ScalarE exp per pair
                budget = 3 if st < 3 else 2
                for _ in range(budget):
                    if bg:
                        bg.popleft()[1]()
            while bg:
                bg.popleft()[1]()

    nc.compile()
    return nc


def _get_nc():
    if "nc" not in _CACHE:
        _CACHE["nc"] = _build()
    return _CACHE["nc"]


def kernel(x, Wq, Wk, Wv, _trace=False):
    x = np.asarray(x)
    nc = _get_nc()

    xT = np.ascontiguousarray(x.transpose(0, 2, 1)).astype(BF)   # [B, C, T]
    w = np.concatenate([Wq, Wk, Wv], axis=1).astype(BF)          # [C, 192]
    # pack to device layout [128, (c, k)]: row p, col 192*c+k = w[128c+p, k]
    w2 = np.ascontiguousarray(
        w.reshape(NC_, 128, 192).transpose(1, 0, 2)).reshape(128, NC_ * 192)
    idn = np.eye(65, dtype=BF)

    # Column-rotated supertile blocks in device SBUF layout
    # [st*128 + p, 512*c + j]: core (b, h) sees supertile st with columns
    # rolled left by 256*h, so its key half is always cols [0:256).
    xqs = {}
    for bidx in range(B):
        blocks = xT[bidx].reshape(C, NST, STQ).transpose(1, 0, 2)  # [st,C,q]
        for h in range(2):
            rb = np.roll(blocks, -256 * h, axis=2) if h else blocks
            # [st, C, q] -> [st, p, c, q] -> [st*128, c*512]
            xqs[(bidx, h)] = np.ascontiguousarray(
                rb.reshape(NST, NC_, 128, STQ).transpose(0, 2, 1, 3)
            ).reshape(NST * 128, NC_ * STQ)

    # Masks in permuted query coordinates: query column j of a supertile is
    # global offset (j + 256h) % 512; diag tile d covers keys 256h+128d+r.
    jj = np.arange(STQ)[None, :]
    rr = np.arange(128)[:, None]
    masks = {}
    for h in range(2):
        gq = (jj + 256 * h) % 512
        m0 = (rr <= gq - 256 * h).astype(BF)
        m1 = (rr <= gq - 256 * h - 128).astype(BF)
        masks[h] = np.concatenate([m0, m1], axis=0)

    in_maps = []
    for cid in range(8):
        bidx, h = cid % 4, cid // 4
        in_maps.append({
            "xq": xqs[(bidx, h)],
            "w": w2,
            "msk": masks[h],
            "idn": idn,
        })

    res = bass_utils.run_bass_kernel_spmd(nc, in_maps, core_ids=list(range(8)),
                                          trace=_trace)
    _CACHE["last_results"] = res

    O = np.empty((B, T, D), dtype=np.float32)
    for bidx in range(B):
        U = np.zeros((65, T), dtype=np.float32)
        for h in range(2):
            part = res.results[bidx + 4 * h]["out"]        # [65, T] permuted
            blocks = part.reshape(65, NST, STQ)
            U += np.roll(blocks, 256 * h, axis=2).reshape(65, T)
        O[bidx] = (U[:D] / U[D:D + 1]).T
    return O
